# revision 40
# baseline (speedup 1.0000x reference)
"""BagModel (segment_reduce) Trainium2 kernel.

Computes out = (1/64 * segment_sum(relu(x @ W1 + b1))) @ W2 + b2 for
4096 bags of exactly 64 consecutive rows each, sharded bag-aligned
across 8 NeuronCores (512 bags / 32768 rows per core, weights
replicated, no cross-core communication).

Layout trick: the host permutes each core's x-shard to
    xh[p, k, g*512 + b] = x[b*64 + g, 128*k + p]
so row-group g contains row g of every bag, with the contraction dim D
on partitions.  The per-bag segment-sum then falls out of PSUM matmul
accumulation: the second (W2) matmul accumulates over the 64 row-groups
with start=(g==0)/stop=(g==63), so no explicit reduction pass over h is
ever needed.  The 4 H-slices of the W2 matmul go to 4 distinct PE
column-groups (tile_position) so they overlap in the array.
"""

import numpy as np

import concourse.bass as bass
import concourse.tile as tile
from concourse import bacc, mybir

N, D, H, C = 262144, 256, 512, 10
N_BAGS, BAG_SIZE = 4096, 64
N_CORES = 8
R = N // N_CORES            # rows per core
BPC = N_BAGS // N_CORES     # bags per core == free dim of each row-group
KT = D // 128               # contraction tiles (2)
MT = H // 128               # H tiles (4)

F32 = mybir.dt.float32
BF16 = mybir.dt.bfloat16
AF = mybir.ActivationFunctionType
ALU = mybir.AluOpType

# compute dtype for the matmul operands: bf16 is 4x faster on the PE
# (fp32 lowers to 2 LOW/HIGH passes at 2 cycles/elem); accumulation
# stays fp32 in PSUM either way
CDT = BF16


def build(nc: bass.Bass, bag: int = BAG_SIZE, bpc: int = BPC, cdt=None):
    """Emit the per-core program.  bag = rows per bag (= number of
    row-groups), bpc = bags per core (= free dim, <= 512)."""
    if cdt is None:
        cdt = CDT
    r = bag * bpc
    xT = nc.declare_dram_parameter("xh", [128, KT, r], cdt, isOutput=False)
    w1 = nc.declare_dram_parameter("w1h", [128, KT, H], cdt, isOutput=False)
    b1 = nc.declare_dram_parameter("b1h", [128, MT], F32, isOutput=False)
    w2 = nc.declare_dram_parameter("w2h", [128, MT, C], cdt, isOutput=False)
    b2 = nc.declare_dram_parameter("b2h", [C, 1], F32, isOutput=False)
    out = nc.declare_dram_parameter("out", [C, bpc], F32, isOutput=True)

    with tile.TileContext(nc) as tc:
        with (
            tc.tile_pool(name="const", bufs=1) as cpool,
            # bufs=8 matches the 8-queue HWDGE rotation: slot reuse then
            # pairs WAW deps on the same queue (implicit FIFO, no extra
            # sync wait — walrus allows only one non-self wait per inst)
            tc.tile_pool(name="xin", bufs=8) as xpool,
            tc.tile_pool(name="hrelu", bufs=3) as hpool,
            tc.tile_pool(name="fin", bufs=1) as fpool,
            tc.tile_pool(name="ps_ht", bufs=6, space="PSUM") as pspool,
            tc.tile_pool(name="ps_out", bufs=1, space="PSUM") as popool,
        ):
            # const loads go on the Activation HWDGE queue so the SP queue
            # starts issuing the (latency-critical) first x tiles at once;
            # w1 split per k-half across queues to halve its arrival time
            # (the first main matmul gates on it)
            w1_sb = cpool.tile([128, KT, H], cdt)
            for k in range(KT):
                nsplit = 2 if k == 0 else 1
                for q in range(nsplit):
                    hh = H // nsplit
                    nc.scalar.dma_start(
                        out=w1_sb[:, k, q * hh:(q + 1) * hh],
                        in_=w1[:, k, q * hh:(q + 1) * hh],
                    )
            b1_sb = cpool.tile([128, MT], F32)
            nc.scalar.dma_start(out=b1_sb[:], in_=b1[:])
            w2_sb = cpool.tile([128, MT, C], cdt)
            nc.scalar.dma_start(out=w2_sb[:], in_=w2[:])
            b2_sb = cpool.tile([C, 1], F32)
            nc.scalar.dma_start(out=b2_sb[:], in_=b2[:])

            # bag-sum accumulators: col-group m holds partial (over H slice
            # m) of out.T at partitions [32m, 32m+10).  The has_written
            # clear of start=True is per-partition (HW-verified), so two
            # col-groups can share a bank on disjoint partitions: {0,2} in
            # bank A, {1,3} in bank B — freeing 2 banks for ht pipelining.
            out_ps_banks = [
                popool.tile([128, bpc], F32, tag=f"outb{b}",
                            name=f"out_psb{b}")
                for b in range(2)
            ]
            out_ps = [out_ps_banks[m % 2] for m in range(MT)]

            # HAM pre-warm: the PE sits idle ~5µs waiting for the first
            # DMA; ~3.5us of dummy matmuls on a memset tile flip the PE
            # clock gate to 8/8 (2.4 GHz) before the real matmuls arrive,
            # which otherwise run the first ~3.4us at 1.2 GHz
            hamw = cpool.tile([128, bpc], cdt, name="hamw")
            nc.gpsimd.memset(hamw[:], 0)
            ham_ps = pspool.tile([128, bpc], F32, tag="ht", name="ham_ps")
            for _ in range(8):
                nc.tensor.matmul(
                    ham_ps[:], lhsT=hamw[:, 0:128], rhs=hamw[:],
                    start=True, stop=True,
                )

            # software pipeline: W2 matmuls run one row-group behind the
            # W1 matmuls, so PE never stalls waiting for the ReLU results
            # (ReLU of group g overlaps the main matmuls of group g+1);
            # the 4 W2 matmuls are emitted back-to-back into 4 distinct PE
            # column groups so they overlap in the array.
            def emit_w2(gprev, htr_prev):
                for m in range(MT):
                    nc.tensor.matmul(
                        out_ps[m][32 * m:32 * m + C, :],
                        lhsT=w2_sb[:, m, :],
                        rhs=htr_prev[m][:],
                        start=(gprev == 0),
                        stop=(gprev == bag - 1),
                        tile_position=(0, 32 * m),
                        skip_group_check=True,
                    )

            prev = None
            for g in range(bag):
                # one tile + dma_start per k-half: the first matmul of the
                # group only waits for its own half (halves head latency,
                # doubles queue parallelism) and keeps 1 sync wait per MM.
                # The very first group is further quarter-split so its
                # transfers parallelize across queues (cuts head latency).
                xks = []
                for k in range(KT):
                    xkk = xpool.tile([128, bpc], cdt, tag=f"xk{k}",
                                     name=f"xk_{g}_{k}")
                    if g == 0:
                        half = bpc // 2
                        for q in range(2):
                            nc.sync.dma_start(
                                out=xkk[:, q * half:(q + 1) * half],
                                in_=xT[:, k, g * bpc + q * half:
                                       g * bpc + (q + 1) * half],
                            )
                    else:
                        nc.sync.dma_start(
                            out=xkk[:],
                            in_=xT[:, k, g * bpc:(g + 1) * bpc],
                        )
                    xks.append(xkk)
                htrs = []
                for m in range(MT):
                    ht = pspool.tile([128, bpc], F32, tag="ht")
                    for k in range(KT):
                        nc.tensor.matmul(
                            ht[:],
                            lhsT=w1_sb[:, k, 128 * m:128 * (m + 1)],
                            rhs=xks[k][:],
                            start=(k == 0),
                            stop=(k == KT - 1),
                        )
                    htr = hpool.tile([128, bpc], cdt, tag=f"htr{m}",
                                     name=f"htr_{g}_{m}")
                    if m < 2:
                        nc.scalar.activation(
                            htr[:], ht[:], AF.Relu,
                            bias=b1_sb[:, m:m + 1], scale=1.0,
                        )
                    else:
                        nc.vector.tensor_scalar(
                            out=htr[:], in0=ht[:],
                            scalar1=b1_sb[:, m:m + 1], scalar2=0.0,
                            op0=ALU.add, op1=ALU.max,
                        )
                    htrs.append(htr)
                if prev is not None:
                    emit_w2(g - 1, prev)
                prev = htrs
            emit_w2(bag - 1, prev)

            # combine the 4 partials + b2 (each op may read only one PSUM
            # operand; b2 fused into the first op)
            acc = fpool.tile([C, bpc], F32, tag="acc")
            nc.vector.tensor_scalar(
                out=acc[:], in0=out_ps[0][0:C, :], scalar1=b2_sb[:],
                scalar2=None, op0=ALU.add,
            )
            for m in range(1, MT):
                nc.vector.tensor_add(
                    acc[:], acc[:], out_ps[m][32 * m:32 * m + C, :])
            nc.sync.dma_start(out=out[:], in_=acc[:])


def _np_cdt(cdt=None):
    if cdt is None:
        cdt = CDT
    if cdt == BF16:
        import ml_dtypes
        return ml_dtypes.bfloat16
    return np.float32


def host_prep_shared(W1, b1, W2, b2, bag=BAG_SIZE, cdt=None):
    np_cdt = _np_cdt(cdt)
    w1h = np.ascontiguousarray(
        W1.reshape(KT, 128, H).transpose(1, 0, 2)).astype(np_cdt)
    b1h = np.ascontiguousarray(
        b1.reshape(MT, 128).T).astype(np.float32)
    w2h = np.ascontiguousarray(
        (W2 / bag).reshape(MT, 128, C).transpose(1, 0, 2)).astype(np_cdt)
    b2h = np.ascontiguousarray(b2.reshape(C, 1)).astype(np.float32)
    return {"w1h": w1h, "b1h": b1h, "w2h": w2h, "b2h": b2h}


def host_prep_x(xs, bag=BAG_SIZE, cdt=None):
    np_cdt = _np_cdt(cdt)
    """xs: [r, D] rows of one core -> xh [128, KT, r] permuted."""
    r = xs.shape[0]
    bpc = r // bag
    xh = xs.reshape(bpc, bag, KT, 128).transpose(3, 2, 1, 0).reshape(128, KT, r)
    return np.ascontiguousarray(xh).astype(np_cdt)


_BUILT = None


def _get_built():
    global _BUILT
    if _BUILT is None:
        nc = bacc.Bacc("TRN2")
        build(nc)
        nc.compile()
        _BUILT = nc
    return _BUILT


def run(x, W1, b1, W2, b2, ids=None, trace=False):
    from concourse.bass_utils import run_bass_kernel_spmd

    nc = _get_built()
    shared = host_prep_shared(W1, b1, W2, b2)
    in_maps = []
    for c in range(N_CORES):
        xs = np.asarray(x[c * R:(c + 1) * R])
        in_maps.append({"xh": host_prep_x(xs), **shared})
    res = run_bass_kernel_spmd(
        nc, in_maps, core_ids=list(range(N_CORES)), trace=trace
    )
    outs = [res.results[c]["out"] for c in range(N_CORES)]
    full = np.concatenate([o.T for o in outs], axis=0).astype(np.float32)
    return full, res


def kernel(x, W1, b1, W2, b2, ids=None):
    full, _ = run(x, W1, b1, W2, b2, ids)
    return full
